# revision 1
# baseline (speedup 1.0000x reference)
"""CapsuleLayer (dynamic routing) Trainium2 kernel — 8 NeuronCores.

Strategy: shard over input capsules I (2048 -> 256/core). W-load drops to
8 MB/core (bf16: 4 MB). Routing softmax/logit state is per-(b, i, j) and thus
core-local; the three routing reductions s_r = sum_i c*u_hat are computed as
per-core partials on the tensor engine and AllReduce'd (128 KB) across cores.

Per-core pipeline:
  P1  u_hat einsum: block-diag(x) [128,128] @ W-octet [128,512] matmuls,
      PSUM -> bf16 SBUF, rearranged to [i-partition, (b, j*k)] via DRAM bounce.
  P2  round 0: uniform-c weighted sums on PE -> diag-extract -> AllReduce ->
      squash(v0) computed redundantly on every core.
  P3  rounds 1,2: b-logit update on vector engine (TT mul + segment reduce),
      softmax (ACT exp + reciprocal), c-weighted sums on PE (col-tiled 4x),
      AllReduce, squash. Round 2's v is the output.
"""
import numpy as np
import ml_dtypes
from contextlib import ExitStack

import concourse.bass as bass
import concourse.mybir as mybir
import concourse.tile as tile
from concourse import bacc
from concourse import bass_utils

B, I, D, J, Kd = 64, 2048, 16, 32, 16
NCORES = 8
IC = I // NCORES      # 256 input capsules per core
NCH = 2               # i-chunks of 128 per core
NOCT = 16             # octets of 8 i per chunk
NSUB = 4              # sub-batches of b
BS = B // NSUB        # 16
JK = J * Kd           # 512
EPS = 1e-7
USE_FOLD = False
BF16 = mybir.dt.bfloat16
F32 = mybir.dt.float32
AX = mybir.AxisListType
OP = mybir.AluOpType
ACTF = mybir.ActivationFunctionType


def _host_prep(inputs, W, core):
    """Per-core DMA-ready layouts (bf16)."""
    Wc = W[core * IC:(core + 1) * IC]  # [256, 32, 16, 16] = [i, j, d, k]
    # wl[ch, oct, (il, d), (j, k)]
    wl = Wc.reshape(NCH, NOCT, 8, J, D, Kd).transpose(0, 1, 2, 4, 3, 5) \
           .reshape(NCH, NOCT, 128, JK)
    wl = np.ascontiguousarray(wl).astype(ml_dtypes.bfloat16)

    xc = inputs[:, core * IC:(core + 1) * IC, :]  # [64, 256, 16] = [b, i, d]
    # xr[ch, oct, sub, il, d, bs]
    xr = xc.reshape(NSUB, BS, NCH, NOCT, 8, D).transpose(2, 3, 0, 4, 5, 1)
    xbd = np.zeros((NCH, NOCT, NSUB, 128, 128), np.float32)
    for il in range(8):
        # rows (il,d) = il*16+d ; cols m = bs*8+il
        xbd[:, :, :, il * 16:(il + 1) * 16, il::8] = xr[:, :, :, il]
    return wl, xbd.astype(ml_dtypes.bfloat16)


def _host_bd16():
    # ones-blockdiag lhsT for the fused s0 reduction: bd16[(bs*8+il), bs'] =
    # (1/J) * (bs == bs')  -> psum[bs', jk] = (1/J) sum_il tmp[(bs,il), jk]
    bd = np.zeros((128, BS), np.float32)
    for bs in range(BS):
        bd[bs * 8:(bs + 1) * 8, bs] = 1.0 / J
    return bd.astype(ml_dtypes.bfloat16)


def _squash_emit(nc, pool, tiny, src_ap, out_dtype, nb=B):
    """Emit squash on s tile [nb, 512] fp32 view [nb, 32, 16]; returns v tile."""
    sq = pool.tile([nb, JK], F32, tag="sq")
    nc.vector.tensor_mul(sq[:], src_ap, src_ap)
    nn = tiny.tile([nb, J], F32, tag="nn")
    nc.vector.tensor_reduce(nn[:], sq[:].rearrange("b (j k) -> b j k", k=Kd),
                            axis=AX.X, op=OP.add)
    t1 = tiny.tile([nb, J], F32, tag="t1")
    nc.vector.tensor_scalar_add(t1[:], nn[:], 1.0)
    t2 = tiny.tile([nb, J], F32, tag="t2")
    nc.vector.tensor_scalar_add(t2[:], nn[:], EPS)
    st = tiny.tile([nb, J], F32, tag="st")
    nc.scalar.sqrt(st[:], t2[:])
    den = tiny.tile([nb, J], F32, tag="den")
    nc.vector.tensor_mul(den[:], t1[:], st[:])
    rden = tiny.tile([nb, J], F32, tag="rden")
    nc.vector.reciprocal(rden[:], den[:])
    sc = tiny.tile([nb, J], F32, tag="sc")
    nc.vector.tensor_mul(sc[:], nn[:], rden[:])
    v = pool.tile([nb, JK], out_dtype, tag="vout")
    nc.vector.tensor_mul(
        v[:].rearrange("b (j k) -> b j k", k=Kd),
        src_ap.rearrange("b (j k) -> b j k", k=Kd),
        sc[:, :, None].broadcast_to([nb, J, Kd]))
    return v


def build_program(collectives=True):
    nc = bacc.Bacc("TRN2", target_bir_lowering=False, debug=False,
                   num_devices=NCORES if collectives else 1)
    wl_d = nc.dram_tensor("wl", [NCH, NOCT, 128, JK], BF16, kind="ExternalInput")
    xbd_d = nc.dram_tensor("xbd", [NCH, NOCT, NSUB, 128, 128], BF16,
                           kind="ExternalInput")
    bd16_d = nc.dram_tensor("bd16", [128, BS], BF16, kind="ExternalInput")
    out_d = nc.dram_tensor("out", [B // NCORES, J, Kd], F32, kind="ExternalOutput")

    with tile.TileContext(nc) as tc, ExitStack() as ctx:
        dram = ctx.enter_context(tc.tile_pool(name="dram", bufs=1, space="DRAM"))
        wpool = ctx.enter_context(tc.tile_pool(name="wp", bufs=3))
        xpool = ctx.enter_context(tc.tile_pool(name="xp", bufs=4))
        epsum = ctx.enter_context(tc.tile_pool(name="ep", bufs=2, space="PSUM"))
        s0psum = ctx.enter_context(tc.tile_pool(name="s0p", bufs=1, space="PSUM"))
        spsum = ctx.enter_context(tc.tile_pool(name="sp", bufs=2, space="PSUM"))
        tmpp = ctx.enter_context(tc.tile_pool(name="tm", bufs=2))
        drp = ctx.enter_context(tc.tile_pool(name="drp", bufs=2))
        uhp = ctx.enter_context(tc.tile_pool(name="uh", bufs=1))
        rp = ctx.enter_context(tc.tile_pool(name="rp", bufs=3))
        smp = ctx.enter_context(tc.tile_pool(name="smp", bufs=5))
        tiny = ctx.enter_context(tc.tile_pool(name="ty", bufs=4))
        vp = ctx.enter_context(tc.tile_pool(name="vp", bufs=1))

        ub = dram.tile([NCH, NOCT, NSUB, 128, JK], BF16)
        sstage = dram.tile([B, J, JK], F32)
        arin = dram.tile([B, J, Kd], F32)
        arout = dram.tile([B, J, Kd], F32)
        vd = dram.tile([B, JK], BF16)

        u_hat = [uhp.tile([128, B, JK], BF16, tag=f"uh{c}", name=f"u_hat{c}")
                 for c in range(NCH)]
        bb = [uhp.tile([128, B, J], F32, tag=f"bb{c}", name=f"bb{c}")
              for c in range(NCH)]

        bd16 = rp.tile([128, BS], BF16, tag="bd16")
        nc.sync.dma_start(bd16[:], bd16_d[:])

        # ---------------- P1: einsum + fused s0 partials ----------------
        s0ps = [s0psum.tile([BS, JK], F32, tag=f"s0p{s}", name=f"s0ps{s}")
                for s in range(NSUB)]
        for ch in range(NCH):
            for oc in range(NOCT):
                wt = wpool.tile([128, JK], BF16)
                nc.sync.dma_start(wt[:], wl_d[ch, oc])
                xt4 = xpool.tile([128, NSUB * 128], BF16)
                xb = xbd_d[ch, oc]  # [NSUB, 128, 128]
                xsrc = bass.AP(tensor=xb.tensor, offset=xb.offset,
                               ap=[[128, 128], [128 * 128, NSUB], [1, 128]])
                nc.sync.dma_start(xt4[:], xsrc)
                tm4 = tmpp.tile([128, NSUB * JK], BF16)
                for sub in range(NSUB):
                    pe = epsum.tile([128, JK], F32)
                    nc.tensor.matmul(pe[:], xt4[:, sub * 128:(sub + 1) * 128],
                                     wt[:], start=True, stop=True)
                    tm = tm4[:, sub * JK:(sub + 1) * JK]
                    if sub % 2 == 0:
                        nc.scalar.copy(tm, pe[:])
                    else:
                        nc.vector.tensor_copy(tm, pe[:])
                    # fused s0 partial: psum[bs,jk] += (1/J) sum_il tm[(bs,il),jk]
                    nc.tensor.matmul(s0ps[sub][:], bd16[:], tm,
                                     start=(ch == 0 and oc == 0),
                                     stop=(ch == NCH - 1 and oc == NOCT - 1))
                if USE_FOLD:
                    # direct sbuf->sbuf partition fold: for each bs, move the
                    # 8 partitions (bs*8+il) into u_hat[oc*8+il] at b=sub*16+bs
                    tview = tm4[:].rearrange("p (s f) -> p s f", f=JK)
                    uview = u_hat[ch][oc * 8:(oc + 1) * 8, :, :]
                    for bs in range(BS):
                        nc.sync.dma_start(
                            uview[:, bs::BS, :],
                            tview[bs * 8:(bs + 1) * 8, :, :])
                else:
                    base = ub[:]
                    blk = (ch * NOCT + oc) * NSUB * 128 * JK
                    udst = bass.AP(tensor=base.tensor, offset=base.offset + blk,
                                   ap=[[JK, 128], [128 * JK, NSUB], [1, JK]])
                    nc.sync.dma_start(udst, tm4[:])
                    # readback -> u_hat[ch][oc*8+il, (sub,bs), :]
                    usrc = bass.AP(tensor=base.tensor, offset=base.offset + blk,
                                   ap=[[JK, 8], [128 * JK, NSUB], [8 * JK, BS], [1, JK]])
                    nc.sync.dma_start(u_hat[ch][oc * 8:(oc + 1) * 8, :, :], usrc)
        # drain s0: sstage[b, j', jk] needs [32, 512] per b; s0 psum rows are
        # identical across j' only for the diag trick -- instead write the
        # 512-vector straight to the diag target: arin[b, j, k] = s0[b, j*16+k].
        for sub in range(NSUB):
            s0sb = drp.tile([BS, JK], F32, tag="s0sb", name=f"s0sb{sub}")
            nc.scalar.copy(s0sb[:], s0ps[sub][:])
            nc.sync.dma_start(
                arin[:].rearrange("b j k -> b (j k)")[sub * BS:(sub + 1) * BS, :],
                s0sb[:])

        rsout = dram.tile([B // NCORES, J, Kd], F32)

        def all_reduce(last=False):
            if collectives:
                if last:
                    nc.gpsimd.collective_compute(
                        "ReduceScatter", OP.add,
                        replica_groups=[list(range(NCORES))],
                        ins=[arin.opt()], outs=[rsout.opt()])
                else:
                    nc.gpsimd.collective_compute(
                        "AllReduce", OP.add,
                        replica_groups=[list(range(NCORES))],
                        ins=[arin.opt()], outs=[arout.opt()])
            else:
                if last:
                    nc.sync.dma_start(rsout[:],
                                      arin[:][0:B // NCORES])
                else:
                    nc.sync.dma_start(arout[:], arin[:])

        # ---------------- rounds ----------------
        for r in range(3):
            all_reduce(last=(r == 2))
            if r < 2:
                sv = vp.tile([B, JK], F32, tag="sv", name=f"sv{r}")
                nc.sync.dma_start(sv[:], arout[:].rearrange("b j k -> b (j k)"))
                v = _squash_emit(nc, vp, tiny, sv[:], BF16)
                nc.sync.dma_start(vd[:], v[:])
            else:
                svs = vp.tile([B // NCORES, JK], F32, tag="svs", name="svs")
                nc.sync.dma_start(svs[:], rsout[:].rearrange("b j k -> b (j k)"))
                v = _squash_emit(nc, vp, tiny, svs[:], F32, nb=B // NCORES)
                nc.sync.dma_start(out_d[:].rearrange("b j k -> b (j k)"), v[:])
                break

            # next round: bb update + softmax + weighted sums, batched by
            # groups of 4 consecutive b
            for g in range(B // 4):
                ps = spsum.tile([128, JK], F32, tag="spsum", name=f"sp{r}_{g}")
                vb4 = rp.tile([128, 4 * JK], BF16, tag="vb4", name=f"vb{r}_{g}")
                vsrc = vd[:]
                vap = bass.AP(tensor=vsrc.tensor, offset=vsrc.offset + g * 4 * JK,
                              ap=[[0, 128], [JK, 4], [1, JK]])
                nc.sync.dma_start(vb4[:], vap)
                for ch in range(NCH):
                    prod = rp.tile([128, 4 * JK], BF16, tag="prod",
                                   name=f"pr{r}_{g}_{ch}")
                    eng = nc.vector if ch == 0 else nc.gpsimd
                    eng.tensor_mul(prod[:],
                                   u_hat[ch][:, g * 4:(g + 1) * 4, :]
                                   .rearrange("p b f -> p (b f)"),
                                   vb4[:])
                    bbs = bb[ch][:, g * 4:(g + 1) * 4, :]  # [128, 4, 32]
                    if r == 0:
                        with nc.allow_low_precision("bb accum in fp32 out"):
                            nc.vector.tensor_reduce(
                                bbs,
                                prod[:].rearrange("p (bj k) -> p bj k", k=Kd),
                                axis=AX.X, op=OP.add)
                    else:
                        binc = smp.tile([128, 4 * J], F32, tag="binc",
                                       name=f"bi{r}_{g}_{ch}")
                        nc.vector.tensor_reduce(
                            binc[:].rearrange("p (bj o) -> p bj o", o=1)
                            if False else binc[:].rearrange(
                                "p (bj) -> p bj", bj=4 * J),
                            prod[:].rearrange("p (bj k) -> p bj k", k=Kd),
                            axis=AX.X, op=OP.add)
                        nc.gpsimd.tensor_add(
                            bbs.rearrange("p b j -> p (b j)"),
                            bbs.rearrange("p b j -> p (b j)"), binc[:])
                    e4 = smp.tile([128, 4 * J], BF16, tag="e4",
                                 name=f"e{r}_{g}_{ch}")
                    nc.scalar.activation(e4[:],
                                         bbs.rearrange("p b j -> p (b j)"),
                                         ACTF.Exp)
                    z4 = tiny.tile([128, 4], F32, tag="z4", name=f"z{r}_{g}_{ch}")
                    nc.vector.tensor_reduce(
                        z4[:], e4[:].rearrange("p (b j) -> p b j", j=J),
                        axis=AX.X, op=OP.add)
                    rz4 = tiny.tile([128, 4], F32, tag="rz4",
                                    name=f"rz{r}_{g}_{ch}")
                    nc.vector.reciprocal(rz4[:], z4[:])
                    c4 = smp.tile([128, 4 * J], BF16, tag="c4",
                                 name=f"c{r}_{g}_{ch}")
                    nc.vector.tensor_mul(
                        c4[:].rearrange("p (b j) -> p b j", j=J),
                        e4[:].rearrange("p (b j) -> p b j", j=J),
                        rz4[:, :, None].broadcast_to([128, 4, J]))
                    for bq in range(4):
                        b = g * 4 + bq
                        nc.tensor.matmul(ps[bq * 32:(bq + 1) * 32, :],
                                         c4[:, bq * J:(bq + 1) * J],
                                         u_hat[ch][:, b, :],
                                         start=(ch == 0), stop=(ch == 1),
                                         tile_position=(0, bq * 32),
                                         skip_group_check=True)
                sdr = drp.tile([128, JK], F32, tag="sdr", name=f"sd{r}_{g}")
                nc.scalar.copy(sdr[:], ps[:])
                nc.sync.dma_start(sstage[:][g * 4:(g + 1) * 4], sdr[:])
                sbase = sstage[:]
                diag = bass.AP(tensor=sbase.tensor,
                               offset=sbase.offset + g * 4 * J * JK,
                               ap=[[J * JK, 4], [JK + Kd, J], [1, Kd]])
                nc.sync.dma_start(arin[:][g * 4:(g + 1) * 4], diag)

    nc.compile()
    return nc


_NC_CACHE = None


_RUN_CACHE = None


def kernel(inputs, W, routings=3):
    """Full inputs in, full [B, J, K] output out. Shards over I across the
    8 NeuronCores internally; first call compiles and caches the executable."""
    global _NC_CACHE, _RUN_CACHE
    import jax
    from jax.sharding import NamedSharding, PartitionSpec
    inputs = np.asarray(inputs, dtype=np.float32)
    W = np.asarray(W, dtype=np.float32)
    if _NC_CACHE is None:
        _NC_CACHE = build_program()
    nc = _NC_CACHE
    if _RUN_CACHE is None:
        _RUN_CACHE = _build_sharded(nc)
    fn, mesh, in_names, out_names, out_avals, zero_outs = _RUN_CACHE
    per_core = []
    for core in range(NCORES):
        wl, xbd = _host_prep(inputs, W, core)
        per_core.append({"wl": wl, "xbd": xbd, "bd16": _host_bd16()})
    sh = NamedSharding(mesh, PartitionSpec("core"))
    concat_in = [jax.device_put(
        np.concatenate([per_core[c][n] for c in range(NCORES)], axis=0), sh)
        for n in in_names]
    zeros = [jax.device_put(
        np.zeros((NCORES * z.shape[0], *z.shape[1:]), z.dtype), sh)
        for z in zero_outs]
    out = fn(*concat_in, *zeros)
    jax.block_until_ready(out)
    oidx = out_names.index("out")
    return np.asarray(out[oidx]).reshape(B, J, Kd)


# ---------------- timing harness (test-only) ----------------
def _build_sharded(nc):
    """Replicate bass2jax.run_bass_via_pjrt's jit construction, returning
    (fn, in_names, out_names, out_avals, n_params)."""
    import jax
    from jax.sharding import Mesh, PartitionSpec
    from jax.experimental.shard_map import shard_map
    from concourse import bass2jax as b2j
    from concourse.bass2jax import _bass_exec_p, install_neuronx_cc_hook, partition_id_tensor
    install_neuronx_cc_hook()
    partition_name = nc.partition_id_tensor.name if nc.partition_id_tensor else None
    in_names, out_names, out_avals, zero_outs = [], [], [], []
    for alloc in nc.m.functions[0].allocations:
        if not isinstance(alloc, mybir.MemoryLocationSet):
            continue
        name = alloc.memorylocations[0].name
        if alloc.kind == "ExternalInput":
            if name != partition_name:
                in_names.append(name)
        elif alloc.kind == "ExternalOutput":
            out_names.append(name)
            shape = tuple(alloc.tensor_shape)
            dtype = mybir.dt.np(alloc.dtype)
            out_avals.append(jax.core.ShapedArray(shape, dtype))
            zero_outs.append(np.zeros(shape, dtype))
    n_params = len(in_names)
    n_outs = len(out_avals)
    all_in = list(in_names) + list(out_names)
    if partition_name is not None:
        all_in.append(partition_name)
    donate = tuple(range(n_params, n_params + n_outs))

    def _body(*args):
        operands = list(args)
        if partition_name is not None:
            operands.append(partition_id_tensor())
        return tuple(_bass_exec_p.bind(
            *operands, out_avals=tuple(out_avals), in_names=tuple(all_in),
            out_names=tuple(out_names), lowering_input_output_aliases=(),
            sim_require_finite=True, sim_require_nnan=True, nc=nc))

    devices = jax.devices()[:NCORES]
    mesh = Mesh(np.array(devices), ("core",))
    in_specs = (PartitionSpec("core"),) * (n_params + n_outs)
    out_specs = (PartitionSpec("core"),) * n_outs
    fn = jax.jit(shard_map(_body, mesh=mesh, in_specs=in_specs,
                           out_specs=out_specs, check_rep=False),
                 donate_argnums=donate, keep_unused=True)
    return fn, mesh, in_names[:n_params], out_names, out_avals, zero_outs


def timed_run(inputs, W, iters=20):
    """Returns (best_ns, times_ns list, output)."""
    import time, jax
    from jax.sharding import NamedSharding, PartitionSpec
    nc = build_program() if _NC_CACHE is None else _NC_CACHE
    fn, mesh, in_names, out_names, out_avals, zero_outs = _build_sharded(nc)
    per_core = []
    for core in range(NCORES):
        wl, xbd = _host_prep(inputs, W, core)
        per_core.append({"wl": wl, "xbd": xbd, "bd16": _host_bd16()})
    sh = NamedSharding(mesh, PartitionSpec("core"))
    concat_in = [jax.device_put(
        np.concatenate([per_core[c][n] for c in range(NCORES)], axis=0), sh)
        for n in in_names]
    def make_zeros():
        return [jax.device_put(
            np.zeros((NCORES * z.shape[0], *z.shape[1:]), z.dtype), sh)
            for z in zero_outs]
    zsets = [make_zeros() for _ in range(iters + 3)]
    out = None
    times = []
    for it in range(iters + 3):
        t0 = time.perf_counter_ns()
        res = fn(*concat_in, *zsets[it])
        jax.block_until_ready(res)
        dt = time.perf_counter_ns() - t0
        if it >= 3:
            times.append(dt)
        out = res
    out_np = np.asarray(out[0]).reshape(B, J, Kd)
    return min(times), times, out_np



# revision 34
# speedup vs baseline: 1.2710x; 1.2710x over previous
"""CapsuleLayer (dynamic routing) Trainium2 kernel — 8 NeuronCores.

Strategy: shard over input capsules I (2048 -> 256/core). W-load drops to
4 MB/core (bf16). Routing softmax/logit state is per-(b, i, j) and thus
core-local; the three routing reductions s_r = sum_i c*u_hat are computed as
per-core partials on the tensor engine and AllReduce'd (128 KB) across cores.

Per-core pipeline:
  P1  u_hat einsum: block-diag(x) [128,128] @ W-octet [128,512] matmuls,
      PSUM -> bf16 SBUF (copies split DVE/ACT/Pool), rearranged to
      [i-partition, (b, j*k)] via DRAM bounce with fused readbacks.
      Round-0 reduction s0 = mean_i u_hat is fused in as bd16 matmuls.
  P2  rounds 1,2: b-logit update via bf16 TT-mult + TT-add tree (segmented
      k-reduce), slabs split DVE/Pool; softmax in half-batch slabs (exp on
      ACT); c-weighted sums on PE (col-tiled 4x); AllReduce; squash.
"""
import numpy as np
import ml_dtypes
from contextlib import ExitStack

import concourse.bass as bass
import concourse.mybir as mybir
import concourse.tile as tile
from concourse import bacc
from concourse import bass_utils

B, I, D, J, Kd = 64, 2048, 16, 32, 16
NCORES = 8
IC = I // NCORES      # 256 input capsules per core
NCH = 2               # i-chunks of 128 per core
NOCT = 16             # octets of 8 i per chunk
NG4 = 4               # octet groups of 4 per chunk (fused DMA)
NSUB = 4              # sub-batches of b
BS = B // NSUB        # 16
JK = J * Kd           # 512
EPS = 1e-7
BF16 = mybir.dt.bfloat16
F32 = mybir.dt.float32
AX = mybir.AxisListType
OP = mybir.AluOpType
ACTF = mybir.ActivationFunctionType

# round-phase tuning: which of the 32 (g-of-4b, ch) tree slabs go to Pool
# (pattern of period 8 -> 8/32 slabs on gpsimd; Pool is ~3.4x slower/elem)
POOL_PAT = (2, 6)


def _host_prep(inputs, W, core):
    """Per-core DMA-ready layouts (bf16), with 4-octet-fused load tiles.

    The einsum output partition order is (il, bs) and the batch free order is
    bo = bs*NSUB + sub, which makes both the DRAM-bounce write and the
    whole-chunk i-major readback flat 2-dim DMAs.
    """
    Wc = W[core * IC:(core + 1) * IC]  # [256, 32, 16, 16] = [i, j, d, k]
    # wl[ch, g4, (il, d), (oc4, j, k)] : 4 octets side by side in free dim
    wl = Wc.reshape(NCH, NG4, 4, 8, J, D, Kd).transpose(0, 1, 3, 5, 2, 4, 6) \
           .reshape(NCH, NG4, 128, 4 * JK)
    wl = np.ascontiguousarray(wl).astype(ml_dtypes.bfloat16)

    xc = inputs[:, core * IC:(core + 1) * IC, :]  # [64, 256, 16] = [b, i, d]
    # xr[ch, g4, oc4, sub, il, d, bs]
    xr = xc.reshape(NSUB, BS, NCH, NG4, 4, 8, D).transpose(2, 3, 4, 0, 5, 6, 1)
    xbd = np.zeros((NCH, NG4, 4, NSUB, 128, 128), np.float32)
    for il in range(8):
        # rows (il,d) = il*16+d ; cols m = il*16+bs (block-diag 16x16)
        xbd[:, :, :, :, il * 16:(il + 1) * 16, il * 16:(il + 1) * 16] = \
            xr[:, :, :, :, il]
    # fold (oc4, sub) into free dim: [ch, g4, 128, (oc4, sub, 128)]
    xbd = xbd.transpose(0, 1, 4, 2, 3, 5).reshape(NCH, NG4, 128, 4 * NSUB * 128)
    return wl, np.ascontiguousarray(xbd).astype(ml_dtypes.bfloat16)


def _bo_perm():
    """natural-b row index for each bo = bs*NSUB + sub row."""
    bo = np.arange(B)
    return (bo % NSUB) * BS + bo // NSUB


def _host_bd16():
    # ones-blockdiag lhsT for the fused s0 reduction: bd16[(il*16+bs), bs'] =
    # (1/J) * (bs == bs')  -> psum[bs', jk] = (1/J) sum_il tmp[(il,bs), jk]
    bd = np.zeros((128, BS), np.float32)
    for bs in range(BS):
        bd[bs::BS, bs] = 1.0 / J
    return bd.astype(ml_dtypes.bfloat16)


def _squash_emit(nc, pool, tiny, src_ap, out_dtype, nb=B):
    """Emit squash on s tile [nb, 512] fp32 view [nb, 32, 16]; returns v tile."""
    sq = pool.tile([nb, JK], F32, tag="sq")
    nc.vector.tensor_mul(sq[:], src_ap, src_ap)
    nn = tiny.tile([nb, J], F32, tag="nn")
    nc.vector.tensor_reduce(nn[:], sq[:].rearrange("b (j k) -> b j k", k=Kd),
                            axis=AX.X, op=OP.add)
    t1 = tiny.tile([nb, J], F32, tag="t1")
    nc.vector.tensor_scalar_add(t1[:], nn[:], 1.0)
    t2 = tiny.tile([nb, J], F32, tag="t2")
    nc.vector.tensor_scalar_add(t2[:], nn[:], EPS)
    st = tiny.tile([nb, J], F32, tag="st")
    nc.scalar.sqrt(st[:], t2[:])
    den = tiny.tile([nb, J], F32, tag="den")
    nc.vector.tensor_mul(den[:], t1[:], st[:])
    rden = tiny.tile([nb, J], F32, tag="rden")
    nc.vector.reciprocal(rden[:], den[:])
    sc = tiny.tile([nb, J], F32, tag="sc")
    nc.vector.tensor_mul(sc[:], nn[:], rden[:])
    v = pool.tile([nb, JK], out_dtype, tag="vout")
    nc.vector.tensor_mul(
        v[:].rearrange("b (j k) -> b j k", k=Kd),
        src_ap.rearrange("b (j k) -> b j k", k=Kd),
        sc[:, :, None].broadcast_to([nb, J, Kd]))
    return v


def build_program(collectives=True):
    nc = bacc.Bacc("TRN2", target_bir_lowering=False, debug=False,
                   num_devices=NCORES if collectives else 1)
    wl_d = nc.dram_tensor("wl", [NCH, NG4, 128, 4 * JK], BF16,
                          kind="ExternalInput")
    xbd_d = nc.dram_tensor("xbd", [NCH, NG4, 128, 4 * NSUB * 128], BF16,
                           kind="ExternalInput")
    bd16_d = nc.dram_tensor("bd16", [128, BS], BF16, kind="ExternalInput")
    out_d = nc.dram_tensor("out", [B // NCORES, J, Kd], F32, kind="ExternalOutput")

    with tile.TileContext(nc) as tc, ExitStack() as ctx:
        dram = ctx.enter_context(tc.tile_pool(name="dram", bufs=1, space="DRAM"))
        xpool = ctx.enter_context(tc.tile_pool(name="xp", bufs=2))
        epsum = ctx.enter_context(tc.tile_pool(name="ep", bufs=4, space="PSUM"))
        s0psum = ctx.enter_context(tc.tile_pool(name="s0p", bufs=1, space="PSUM"))
        spsum = ctx.enter_context(tc.tile_pool(name="sp", bufs=2, space="PSUM"))
        drp = ctx.enter_context(tc.tile_pool(name="drp", bufs=2))
        uhp = ctx.enter_context(tc.tile_pool(name="uh", bufs=1))
        vbp = ctx.enter_context(tc.tile_pool(name="vb", bufs=2))
        vbq = ctx.enter_context(tc.tile_pool(name="vq", bufs=3))
        trp = ctx.enter_context(tc.tile_pool(name="tr", bufs=3))
        smp = ctx.enter_context(tc.tile_pool(name="smp", bufs=1))
        tiny = ctx.enter_context(tc.tile_pool(name="ty", bufs=2))
        vp = ctx.enter_context(tc.tile_pool(name="vp", bufs=1))

        # ub[ch, oc, il, bs, sub, jk]: i-major bounce staging
        ub = dram.tile([NCH, NOCT, 8, BS, NSUB, JK], BF16)
        sstage = dram.tile([B, J, JK], F32)
        arin = dram.tile([B, J, Kd], F32)
        arout = dram.tile([B, J, Kd], F32)
        vd = dram.tile([B, JK], BF16)

        u_hat = [uhp.tile([128, B, JK], BF16, tag=f"uh{c}", name=f"u_hat{c}")
                 for c in range(NCH)]
        bb = [uhp.tile([128, B, J], F32, tag=f"bb{c}", name=f"bb{c}")
              for c in range(NCH)]

        bd16 = tiny.tile([128, BS], BF16, tag="bd16")
        nc.sync.dma_start(bd16[:], bd16_d[:])

        # ---------------- P1: einsum + fused s0 partials ----------------
        # single s0 psum bank: sub-block rows at 32-partition spacing
        # (tile_position col offsets must be multiples of 32)
        s0ps = s0psum.tile([128, JK], F32, tag="s0p", name="s0ps")
        # NOTE: GPSIMD cannot read PSUM, so copies alternate DVE/ACT only
        cpeng = [nc.vector, nc.scalar]
        ncop = 0
        for ch in range(NCH):
            for g4 in range(NG4):
                wt = vbp.tile([128, 4 * JK], BF16, tag="vb",
                              name=f"wt{ch}_{g4}")
                nc.sync.dma_start(wt[:], wl_d[ch, g4])
                xt = xpool.tile([128, 4 * NSUB * 128], BF16, tag="xt",
                                name=f"xt{ch}_{g4}")
                nc.sync.dma_start(xt[:], xbd_d[ch, g4])
                for o4 in range(4):
                    oc = g4 * 4 + o4
                    tm4 = trp.tile([128, NSUB * JK], BF16, tag="pr",
                                   name=f"tm{ch}_{oc}")
                    for sub in range(NSUB):
                        pe = epsum.tile([128, JK], F32)
                        xsl = xt[:, (o4 * NSUB + sub) * 128:
                                 (o4 * NSUB + sub + 1) * 128]
                        nc.tensor.matmul(pe[:], xsl,
                                         wt[:, o4 * JK:(o4 + 1) * JK],
                                         start=True, stop=True)
                        tm = tm4[:, sub * JK:(sub + 1) * JK]
                        eng = cpeng[ncop % 2]
                        ncop += 1
                        if eng is nc.scalar:
                            eng.copy(tm, pe[:])
                        else:
                            eng.tensor_copy(tm, pe[:])
                        # fused s0 partial:
                        # psum[sub*32+bs,jk] += (1/J) sum_il tm[(bs,il),jk]
                        nc.tensor.matmul(s0ps[sub * 32:sub * 32 + BS, :],
                                         bd16[:], tm,
                                         start=(ch == 0 and oc == 0),
                                         stop=(ch == NCH - 1 and oc == NOCT - 1),
                                         tile_position=(0, sub * 32),
                                         skip_group_check=True)
                    # bounce write: flat [[2048,128],[1,2048]] (4KB descs)
                    base = ub[:]
                    blk = (ch * NOCT + oc) * 8 * BS * NSUB * JK
                    udst = bass.AP(tensor=base.tensor, offset=base.offset + blk,
                                   ap=[[NSUB * JK, 128], [1, NSUB * JK]])
                    nc.sync.dma_start(udst, tm4[:])
        # whole-chunk readbacks in b-halves; ch1's h1 chunk is emitted after
        # the round-0 AR/squash chain below so the small s0-path DMAs don't
        # queue behind it on the DMA engines.
        def readback(ch, bh):
            base = ub[:]
            blk = ch * NOCT * 8 * BS * NSUB * JK
            usrc = bass.AP(tensor=base.tensor,
                           offset=base.offset + blk + bh * (B // 2) * JK,
                           ap=[[BS * NSUB * JK, 128], [1, B * JK // 2]])
            nc.sync.dma_start(u_hat[ch][:, bh * 32:(bh + 1) * 32, :], usrc)

        readback(0, 0)
        readback(0, 1)
        # drain s0 -> arin[bo, jk] rows bo = bs*NSUB + sub
        s0sb = vp.tile([128, JK], F32, tag="sv", name="s0sb")
        nc.scalar.copy(s0sb[:], s0ps[:])
        abase = arin[:]
        for sub in range(NSUB):
            adst = bass.AP(tensor=abase.tensor,
                           offset=abase.offset + sub * JK,
                           ap=[[NSUB * JK, BS], [1, JK]])
            nc.sync.dma_start(adst, s0sb[sub * 32:sub * 32 + BS, :])

        rsout = dram.tile([B // NCORES, J, Kd], F32)

        def all_reduce(last=False):
            if collectives:
                if last:
                    nc.gpsimd.collective_compute(
                        "ReduceScatter", OP.add,
                        replica_groups=[list(range(NCORES))],
                        ins=[arin.opt()], outs=[rsout.opt()])
                else:
                    nc.gpsimd.collective_compute(
                        "AllReduce", OP.add,
                        replica_groups=[list(range(NCORES))],
                        ins=[arin.opt()], outs=[arout.opt()])
            else:
                if last:
                    nc.sync.dma_start(rsout[:], arin[:][0:B // NCORES])
                else:
                    nc.sync.dma_start(arout[:], arin[:])

        # ---------------- rounds ----------------
        # r in {0,1}: AllReduce s_r, squash -> v_r, then bb update (init at
        # r=0, += at r=1), softmax c, s_{r+1} matmuls.  r=2: final AR+squash.
        for r in range(3):
            all_reduce(last=(r == 2))
            if r < 2:
                sv = vp.tile([B, JK], F32, tag="sv", name=f"sv{r}")
                nc.sync.dma_start(sv[:], arout[:].rearrange("b j k -> b (j k)"))
                v = _squash_emit(nc, vp, tiny, sv[:], BF16)
                nc.sync.dma_start(vd[:], v[:])
                if r == 0:
                    readback(1, 0)
                    readback(1, 1)
            else:
                svs = vp.tile([B // NCORES, JK], F32, tag="svs", name="svs")
                nc.sync.dma_start(svs[:], rsout[:].rearrange("b j k -> b (j k)"))
                v = _squash_emit(nc, vp, tiny, svs[:], F32, nb=B // NCORES)
                nc.sync.dma_start(out_d[:].rearrange("b j k -> b (j k)"), v[:])
                break

            # ---- per half-batch: bb tree, softmax, s matmuls ----
            # s-mms of half h run on PE while DVE/Pool chew half h+1's tree.
            def tree_slab(g, ch, slab_idx):
                use_pool = (slab_idx % 8) in POOL_PAT
                eng = nc.gpsimd if use_pool else nc.vector
                uh4 = u_hat[ch][:, g * 4:(g + 1) * 4, :] \
                    .rearrange("p b f -> p (b f)")          # [128, 2048]
                pr = trp.tile([128, 4 * JK], BF16, tag="pr",
                              name=f"pr{r}_{g}_{ch}")
                eng.tensor_mul(pr[:], uh4, vbs[g][:])
                # tree over k in place: 16 -> 8 -> 4 -> 2 -> bb
                prv = pr[:].rearrange("p (bj k) -> p bj k", k=16)
                eng.tensor_add(prv[:, :, 0:8], prv[:, :, 0:8],
                               prv[:, :, 8:16])
                eng.tensor_add(prv[:, :, 0:4], prv[:, :, 0:4],
                               prv[:, :, 4:8])
                eng.tensor_add(prv[:, :, 0:2], prv[:, :, 0:2],
                               prv[:, :, 2:4])
                bbs = bb[ch][:, g * 4:(g + 1) * 4, :] \
                    .rearrange("p b j -> p (b j)")          # [128, 128]
                with nc.allow_low_precision("bb from bf16 tree"):
                    if r == 0:
                        eng.tensor_add(bbs[:, :, None], prv[:, :, 0:1],
                                       prv[:, :, 1:2])
                    else:
                        eng.tensor_add(prv[:, :, 0:1], prv[:, :, 0:1],
                                       prv[:, :, 1:2])
                        eng.tensor_add(bbs[:, :, None], bbs[:, :, None],
                                       prv[:, :, 0:1])

            vbs = {}
            slab_idx = 0
            for h in range(2):
                # vB broadcasts + tree slabs for this half
                for gq in range(8):
                    g = h * 8 + gq
                    vb = vbq.tile([128, 4 * JK], BF16, tag="vb",
                                  name=f"vb{r}_{g}")
                    vsrc = vd[:]
                    vap = bass.AP(tensor=vsrc.tensor,
                                  offset=vsrc.offset + g * 4 * JK,
                                  ap=[[0, 128], [JK, 4], [1, JK]])
                    nc.sync.dma_start(vb[:], vap)
                    vbs[g] = vb
                    for ch in range(NCH):
                        tree_slab(g, ch, slab_idx)
                        slab_idx += 1
                c4t = [None, None]
                for ch in range(NCH):
                    bbh = bb[ch][:, h * 32:(h + 1) * 32, :] \
                        .rearrange("p b j -> p (b j)")          # [128, 1024]
                    e = smp.tile([128, 32 * J], BF16, tag=f"e{ch}",
                                 name=f"e{r}_{h}_{ch}")
                    nc.scalar.activation(e[:], bbh, ACTF.Exp)
                    z = tiny.tile([128, 32], F32, tag=f"z{ch}",
                                  name=f"z{r}_{h}_{ch}")
                    nc.vector.tensor_reduce(
                        z[:], e[:].rearrange("p (b j) -> p b j", j=J),
                        axis=AX.X, op=OP.add)
                    rz = tiny.tile([128, 32], F32, tag=f"rz{ch}",
                                   name=f"rz{r}_{h}_{ch}")
                    nc.vector.reciprocal(rz[:], z[:])
                    ct = smp.tile([128, 32 * J], BF16, tag=f"c{ch}",
                                  name=f"c{r}_{h}_{ch}")
                    nc.vector.tensor_mul(
                        ct[:].rearrange("p (b j) -> p b j", j=J),
                        e[:].rearrange("p (b j) -> p b j", j=J),
                        rz[:, :, None].broadcast_to([128, 32, J]))
                    c4t[ch] = ct
                for gq in range(8):        # 8 groups of 4 b in this half
                    g = h * 8 + gq
                    ps = spsum.tile([128, JK], F32, tag="spsum",
                                    name=f"sp{r}_{g}")
                    for ch in range(NCH):
                        for bq in range(4):
                            b = g * 4 + bq
                            bl = gq * 4 + bq   # b index within half
                            nc.tensor.matmul(
                                ps[bq * 32:(bq + 1) * 32, :],
                                c4t[ch][:, bl * J:(bl + 1) * J],
                                u_hat[ch][:, b, :],
                                start=(ch == 0), stop=(ch == 1),
                                tile_position=(0, bq * 32),
                                skip_group_check=True)
                    sdr = drp.tile([128, JK], F32, tag="sdr", name=f"sd{r}_{g}")
                    nc.scalar.copy(sdr[:], ps[:])
                    nc.sync.dma_start(sstage[:][g * 4:(g + 1) * 4], sdr[:])
                sbase = sstage[:]
                diag = bass.AP(tensor=sbase.tensor,
                               offset=sbase.offset + h * 32 * J * JK,
                               ap=[[J * JK, 32], [JK + Kd, J], [1, Kd]])
                nc.sync.dma_start(arin[:][h * 32:(h + 1) * 32], diag)

    nc.compile()
    return nc


_NC_CACHE = None


_RUN_CACHE = None


def kernel(inputs, W, routings=3):
    """Full inputs in, full [B, J, K] output out. Shards over I across the
    8 NeuronCores internally; first call compiles and caches the executable."""
    global _NC_CACHE, _RUN_CACHE
    import jax
    from jax.sharding import NamedSharding, PartitionSpec
    inputs = np.asarray(inputs, dtype=np.float32)
    W = np.asarray(W, dtype=np.float32)
    if _NC_CACHE is None:
        _NC_CACHE = build_program()
    nc = _NC_CACHE
    if _RUN_CACHE is None:
        _RUN_CACHE = _build_sharded(nc)
    fn, mesh, in_names, out_names, out_avals, zero_outs = _RUN_CACHE
    per_core = []
    for core in range(NCORES):
        wl, xbd = _host_prep(inputs, W, core)
        per_core.append({"wl": wl, "xbd": xbd, "bd16": _host_bd16()})
    sh = NamedSharding(mesh, PartitionSpec("core"))
    concat_in = [jax.device_put(
        np.concatenate([per_core[c][n] for c in range(NCORES)], axis=0), sh)
        for n in in_names]
    zeros = [jax.device_put(
        np.zeros((NCORES * z.shape[0], *z.shape[1:]), z.dtype), sh)
        for z in zero_outs]
    out = fn(*concat_in, *zeros)
    jax.block_until_ready(out)
    oidx = out_names.index("out")
    raw = np.asarray(out[oidx]).reshape(B, J, Kd)
    out_nat = np.empty_like(raw)
    out_nat[_bo_perm()] = raw
    return out_nat


# ---------------- timing harness (test-only) ----------------
def _build_sharded(nc):
    """Replicate bass2jax.run_bass_via_pjrt's jit construction, returning
    (fn, in_names, out_names, out_avals, n_params)."""
    import jax
    from jax.sharding import Mesh, PartitionSpec
    from jax.experimental.shard_map import shard_map
    from concourse import bass2jax as b2j
    from concourse.bass2jax import _bass_exec_p, install_neuronx_cc_hook, partition_id_tensor
    install_neuronx_cc_hook()
    partition_name = nc.partition_id_tensor.name if nc.partition_id_tensor else None
    in_names, out_names, out_avals, zero_outs = [], [], [], []
    for alloc in nc.m.functions[0].allocations:
        if not isinstance(alloc, mybir.MemoryLocationSet):
            continue
        name = alloc.memorylocations[0].name
        if alloc.kind == "ExternalInput":
            if name != partition_name:
                in_names.append(name)
        elif alloc.kind == "ExternalOutput":
            out_names.append(name)
            shape = tuple(alloc.tensor_shape)
            dtype = mybir.dt.np(alloc.dtype)
            out_avals.append(jax.core.ShapedArray(shape, dtype))
            zero_outs.append(np.zeros(shape, dtype))
    n_params = len(in_names)
    n_outs = len(out_avals)
    all_in = list(in_names) + list(out_names)
    if partition_name is not None:
        all_in.append(partition_name)
    donate = tuple(range(n_params, n_params + n_outs))

    def _body(*args):
        operands = list(args)
        if partition_name is not None:
            operands.append(partition_id_tensor())
        return tuple(_bass_exec_p.bind(
            *operands, out_avals=tuple(out_avals), in_names=tuple(all_in),
            out_names=tuple(out_names), lowering_input_output_aliases=(),
            sim_require_finite=True, sim_require_nnan=True, nc=nc))

    devices = jax.devices()[:NCORES]
    mesh = Mesh(np.array(devices), ("core",))
    in_specs = (PartitionSpec("core"),) * (n_params + n_outs)
    out_specs = (PartitionSpec("core"),) * n_outs
    fn = jax.jit(shard_map(_body, mesh=mesh, in_specs=in_specs,
                           out_specs=out_specs, check_rep=False),
                 donate_argnums=donate, keep_unused=True)
    return fn, mesh, in_names[:n_params], out_names, out_avals, zero_outs


def timed_run(inputs, W, iters=20):
    """Returns (best_ns, times_ns list, output)."""
    import time, jax
    from jax.sharding import NamedSharding, PartitionSpec
    nc = build_program() if _NC_CACHE is None else _NC_CACHE
    fn, mesh, in_names, out_names, out_avals, zero_outs = _build_sharded(nc)
    per_core = []
    for core in range(NCORES):
        wl, xbd = _host_prep(inputs, W, core)
        per_core.append({"wl": wl, "xbd": xbd, "bd16": _host_bd16()})
    sh = NamedSharding(mesh, PartitionSpec("core"))
    concat_in = [jax.device_put(
        np.concatenate([per_core[c][n] for c in range(NCORES)], axis=0), sh)
        for n in in_names]
    def make_zeros():
        return [jax.device_put(
            np.zeros((NCORES * z.shape[0], *z.shape[1:]), z.dtype), sh)
            for z in zero_outs]
    zsets = [make_zeros() for _ in range(iters + 3)]
    out = None
    times = []
    for it in range(iters + 3):
        t0 = time.perf_counter_ns()
        res = fn(*concat_in, *zsets[it])
        jax.block_until_ready(res)
        dt = time.perf_counter_ns() - t0
        if it >= 3:
            times.append(dt)
        out = res
    raw = np.asarray(out[0]).reshape(B, J, Kd)
    out_np = np.empty_like(raw)
    out_np[_bo_perm()] = raw
    return min(times), times, out_np
